# revision 20
# baseline (speedup 1.0000x reference)
"""LoRA-MoE layer kernel for 8 Trainium2 NeuronCores.

Math (exact restructure of the reference):
  gates = softmax(x @ M * s'),  M = Wq @ (expert_emb @ Wk).T,  s' = (hd**-0.5)/H
    (mean over heads of blocked QK^T == full 768-dot product / H)
  T = x @ concat(A_i)  (rows, 392); scale expert block i by gates[:, i]
  y = T_scaled @ concat(B_i)  (rows, 2304)
  importance_e = sum gates[:, e];  loss = 2*var(imp, ddof=1)/(mean^2+eps)

Sharding: data-parallel over B*N rows; core c takes rows [c*1024, (c+1)*1024)
(= batch c). All weights replicated. Matmuls run in bf16 (fp32 PSUM accum).
"""

import numpy as np
import ml_dtypes

import concourse.bass as bass
import concourse.bacc as bacc
import concourse.mybir as mybir
import concourse.tile as tile
from concourse import masks
from concourse.bass_utils import run_bass_kernel_spmd

B, N, C = 8, 1024, 768
E = 7
LORA_DIMS = [8, 16, 32, 48, 64, 96, 128]
R = sum(LORA_DIMS)          # 392
OUT = 3 * C                 # 2304
ROWS = N                    # rows per core
H = 4
SCALE = (C // H) ** -0.5 / H
NCHUNKS = ROWS // 128       # 8 row chunks per core
CCH = C // 128              # 6 contraction chunks
KCH = [128, 128, 128, R - 384]   # y-matmul K chunks over 392
NSPLIT = [512, 512, 512, 512, OUT - 2048]  # y-matmul N chunks

BF16 = mybir.dt.bfloat16
F32 = mybir.dt.float32

_cache = {}


def _build():
    nc = bacc.Bacc("TRN2", target_bir_lowering=False, debug=False, num_devices=8)

    xt_d = nc.dram_tensor("xt", [C, ROWS], BF16, kind="ExternalInput")
    wqt_d = nc.dram_tensor("wqt", [C, C], F32, kind="ExternalInput")
    wk_d = nc.dram_tensor("wk", [C, C], F32, kind="ExternalInput")
    embt_d = nc.dram_tensor("embt", [C, E], F32, kind="ExternalInput")
    acat_d = nc.dram_tensor("acat", [C, R], BF16, kind="ExternalInput")
    bcat_d = nc.dram_tensor("bcat", [R, OUT], BF16, kind="ExternalInput")
    y_d = nc.dram_tensor("y", [ROWS, OUT], F32, kind="ExternalOutput")
    imp_d = nc.dram_tensor("imp", [1, NCHUNKS * E], F32, kind="ExternalOutput")

    with tile.TileContext(nc) as tc:
        with (
            tc.tile_pool(name="const", bufs=1) as const_pool,
            tc.tile_pool(name="wts", bufs=1) as wts_pool,
            tc.tile_pool(name="ts", bufs=3) as ts_pool,
            tc.tile_pool(name="tst", bufs=3) as tst_pool,
            tc.tile_pool(name="yb", bufs=3) as yb_pool,
            tc.tile_pool(name="sm", bufs=2) as sm_pool,
            tc.tile_pool(name="pt", bufs=2, space="PSUM") as pt_pool,
            tc.tile_pool(name="ptr", bufs=2, space="PSUM") as ptr_pool,
            tc.tile_pool(name="py", bufs=2, space="PSUM") as py_pool,
        ):
            ident = const_pool.tile([128, 128], BF16)
            masks.make_identity(nc, ident[:])
            ones = const_pool.tile([128, 1], F32)
            nc.gpsimd.memset(ones[:], 1.0)
            gates_all = const_pool.tile([128, NCHUNKS * E], F32)

            # ---- load weights (router weights first: they gate M-prep) ----
            wqt_sb, wk_sb, embt_sb = [], [], []
            for k in range(CCH):
                t = wts_pool.tile([128, E], F32, tag=f"em{k}")
                nc.sync.dma_start(t[:], embt_d[k * 128:(k + 1) * 128, :])
                embt_sb.append(t)
                t = wts_pool.tile([128, C], F32, tag=f"wk{k}")
                nc.sync.dma_start(t[:], wk_d[k * 128:(k + 1) * 128, :])
                wk_sb.append(t)
                t = wts_pool.tile([128, C], F32, tag=f"wqt{k}")
                nc.sync.dma_start(t[:], wqt_d[k * 128:(k + 1) * 128, :])
                wqt_sb.append(t)
            acatm_sb = []
            for k in range(CCH):
                t = wts_pool.tile([128, R + 2 * E], BF16, tag=f"am{k}")
                nc.sync.dma_start(t[:, 0:R], acat_d[k * 128:(k + 1) * 128, :])
                acatm_sb.append(t)
            xt_sb = []
            for k in range(CCH):
                t = wts_pool.tile([128, ROWS], BF16, tag=f"xt{k}")
                nc.sync.dma_start(t[:], xt_d[k * 128:(k + 1) * 128, :])
                xt_sb.append(t)
            bcat_sb = []
            off = 0
            for k, kw in enumerate(KCH):
                t = wts_pool.tile([kw, OUT], BF16, tag=f"bc{k}")
                nc.sync.dma_start(t[:], bcat_d[off:off + kw, :])
                bcat_sb.append(t)
                off += kw

            # ---- M-prep (fp32): KhatT = (emb @ Wk).T then M = Wq @ Khat.T
            # M is split into bf16 hi+lo column pairs so the bf16 fused
            # matmul reproduces the fp32 scores to ~1e-6.
            khatt_sb = []
            for jc in range(CCH):
                ps = ptr_pool.tile([128, E], F32, tag="mprep")
                for cc in range(CCH):
                    nc.tensor.matmul(
                        ps[:], wk_sb[cc][:, jc * 128:(jc + 1) * 128],
                        embt_sb[cc][:], start=(cc == 0), stop=(cc == CCH - 1))
                t = wts_pool.tile([128, E], F32, tag=f"kh{jc}")
                nc.vector.tensor_copy(t[:], ps[:])
                khatt_sb.append(t)
            for ccc in range(CCH):
                ps = ptr_pool.tile([128, E], F32, tag="mprep")
                for jc in range(CCH):
                    nc.tensor.matmul(
                        ps[:], wqt_sb[jc][:, ccc * 128:(ccc + 1) * 128],
                        khatt_sb[jc][:], start=(jc == 0), stop=(jc == CCH - 1))
                am = acatm_sb[ccc]
                nc.vector.tensor_copy(am[:, R:R + E], ps[:])
                nc.vector.tensor_sub(am[:, R + E:R + 2 * E], ps[:], am[:, R:R + E])

            # ---- main loop over 8 row chunks ----
            for r in range(NCHUNKS):
                rs = slice(r * 128, (r + 1) * 128)
                # fused T|scores matmul: (128 rows, 392+7)
                pt = pt_pool.tile([128, R + 2 * E], F32, tag="pt")
                for k in range(CCH):
                    nc.tensor.matmul(
                        pt[:], xt_sb[k][:, rs], acatm_sb[k][:],
                        start=(k == 0), stop=(k == CCH - 1))

                # evict full PSUM row (T | score_hi | score_lo) to SBUF
                tfull = ts_pool.tile([128, R + 2 * E], F32, tag="tfull")
                nc.vector.tensor_copy(tfull[:], pt[:])

                # softmax over E=7 (no max-sub needed; |scores*s'| < ~4)
                sc7 = sm_pool.tile([128, E], F32, tag="sc7")
                nc.vector.tensor_add(sc7[:], tfull[:, R:R + E],
                                     tfull[:, R + E:R + 2 * E])
                eg = sm_pool.tile([128, E], F32, tag="eg")
                gsum = sm_pool.tile([128, 1], F32, tag="gsum")
                nc.scalar.activation(
                    eg[:], sc7[:], mybir.ActivationFunctionType.Exp,
                    scale=SCALE, accum_out=gsum[:])
                grec = sm_pool.tile([128, 1], F32, tag="grec")
                nc.vector.reciprocal(grec[:], gsum[:])
                gcol = slice(r * E, (r + 1) * E)
                nc.vector.tensor_scalar_mul(gates_all[:, gcol], eg[:], grec[:])

                # scale expert blocks by gates, cast to bf16
                ts = ts_pool.tile([128, R], BF16, tag="ts")
                o = 0
                for i, d in enumerate(LORA_DIMS):
                    nc.vector.tensor_scalar_mul(
                        ts[:, o:o + d], tfull[:, o:o + d],
                        gates_all[:, r * E + i:r * E + i + 1])
                    o += d

                # transpose Ts (128,392) -> TsT blocks in one PSUM bank
                ptr = ptr_pool.tile([128, 512], BF16, tag="ptr")
                o = 0
                for k, kw in enumerate(KCH):
                    nc.tensor.transpose(
                        ptr[:kw, k * 128:k * 128 + 128],
                        ts[:, o:o + kw], ident[:])
                    o += kw
                tst = tst_pool.tile([128, 512], BF16, tag="tst")
                nc.vector.tensor_copy(tst[:, 0:384], ptr[:, 0:384])
                nc.vector.tensor_copy(tst[:KCH[3], 384:512], ptr[:KCH[3], 384:512])

                # y matmul: accumulate over 4 K chunks for each N chunk
                ybt = yb_pool.tile([128, OUT], F32, tag="yb")
                no = 0
                for nw in NSPLIT:
                    py = py_pool.tile([128, nw], F32, tag="py")
                    for k, kw in enumerate(KCH):
                        nc.tensor.matmul(
                            py[:], tst[:kw, k * 128:k * 128 + 128],
                            bcat_sb[k][:, no:no + nw],
                            start=(k == 0), stop=(k == len(KCH) - 1))
                    nc.any.tensor_copy(ybt[:, no:no + nw], py[:])
                    no += nw
                nc.sync.dma_start(y_d[rs, :], ybt[:])

            # ---- importance partial: column sums of gates_all ----
            pimp = ptr_pool.tile([1, NCHUNKS * E], F32, tag="mprep")
            nc.tensor.matmul(pimp[:], ones[:], gates_all[:], start=True, stop=True)
            imp_sb = sm_pool.tile([1, NCHUNKS * E], F32, tag="imp")
            nc.vector.tensor_copy(imp_sb[:], pimp[:])
            nc.sync.dma_start(imp_d[:], imp_sb[:])

    nc.compile()
    return nc


def kernel(**inputs):
    x = np.asarray(inputs["x"], np.float32)
    Wq = np.asarray(inputs["Wq"], np.float32)
    Wk = np.asarray(inputs["Wk"], np.float32)
    emb = np.asarray(inputs["expert_emb"], np.float32)
    As = [np.asarray(inputs[f"A{i}"], np.float32) for i in range(E)]
    Bs = [np.asarray(inputs[f"B{i}"], np.float32) for i in range(E)]

    bf = ml_dtypes.bfloat16
    wqt = np.ascontiguousarray(Wq.T)
    wk = np.ascontiguousarray(Wk)
    embt = np.ascontiguousarray(emb.T)
    acat = np.concatenate(As, axis=1).astype(bf)
    bcat = np.concatenate(Bs, axis=0).astype(bf)

    xf = x.reshape(B * N, C)
    in_maps = []
    for c in range(8):
        xt = np.ascontiguousarray(xf[c * ROWS:(c + 1) * ROWS].T).astype(bf)
        in_maps.append(dict(xt=xt, wqt=wqt, wk=wk, embt=embt,
                            acat=acat, bcat=bcat))

    if "nc" not in _cache:
        _cache["nc"] = _build()
    res = run_bass_kernel_spmd(_cache["nc"], in_maps, list(range(8)))
    if inputs.get("_profile"):
        import time as _t
        t0 = _t.time()
        res = run_bass_kernel_spmd(_cache["nc"], in_maps, list(range(8)))
        dt = _t.time() - t0
        print(f"HW exec time: {dt*1e9:.0f} ns (warm dispatch wall)")

    y = np.empty((B * N, OUT), np.float32)
    imp = np.zeros(E, np.float64)
    for c in range(8):
        out = res.results[c]
        y[c * ROWS:(c + 1) * ROWS] = out["y"]
        imp += out["imp"].reshape(NCHUNKS, E).sum(axis=0)
    yf = y.reshape(B, N, OUT)
    eps = 1e-10
    cv2 = np.var(imp, ddof=1) / (imp.mean() ** 2 + eps)
    loss = np.float32(2.0 * cv2)
    return (yf, loss)
